# revision 2
# baseline (speedup 1.0000x reference)
"""Trainium2 Bass kernel for nn_BridgeNetworkKAN.

Math (per batch row b):
  x = concat(state, novelU)                                   [128]
  bases[i,c] = cubic B-spline basis of x_i on uniform grid    [128, 8]
  kan = silu(x) @ scale_base + einsum(bases, scale_sp*coef)   [1024]
  bias = kan @ bias_w (+ bias_b)                              [64]
  AB = (kan @ f_w (+ f_b)).reshape(64, 128)                   [64, 128]
  out = AB @ x + bias                                         [64]

Key tricks:
  * Uniform-knot B-spline bases via 4th finite difference of relu((t-j)/c)^3
    (c = cbrt(6), t = 2.5*x + 5.5): 12 ACT relus + 62 DVE TT ops per
    [128, 512] tile instead of the Cox-de-Boor recursion.
  * All matmuls in float32r (full PE rate for N=512; rel err ~2e-4).
  * Activations kept feature-on-partition (xT) so every contraction is a
    natural PE matmul; AB computed batch-on-partition so the 64 MB AB
    output DMAs out contiguously.
  * bmm (out = AB @ x) fused into the AB epilogue: 4 scalar_tensor_tensor
    ops per PSUM bank (multiply by x, accumulate-reduce over u) while the
    bank drains to SBUF on the scalar engine.

Sharding: data-parallel over batch, 16384/8 = 2048 rows per core;
all parameters replicated.
"""
import sys

if "/opt/trn_rl_repo" not in sys.path:
    sys.path.insert(0, "/opt/trn_rl_repo")

import numpy as np

import concourse.bass as bass
import concourse.mybir as mybir
import concourse.tile as tile
from concourse import bacc, bass_utils
from concourse.masks import make_identity

F32 = mybir.dt.float32
F32R = mybir.dt.float32r
AF = mybir.ActivationFunctionType
ALU = mybir.AluOpType

N_CORES = 8
B_FULL = 16384
B_L = B_FULL // N_CORES        # 2048 rows per core
CH = 512                       # batch chunk (matmul moving N)
NCH = B_L // CH                # 4
NBT = B_L // 128               # 16 batch tiles of 128
I = 128                        # input features (STATE + N_U)
O = 1024                       # KAN hidden
NOT = O // 128                 # 8 o-tiles
S = 64                         # STATE
SU = S * I                     # 8192
NSC = SU // CH                 # 16 su-chunks of 512

CBRT6 = float(np.cbrt(6.0))
R_SCALE = float(np.float32(2.5 / CBRT6))


def _build():
    nc = bacc.Bacc("TRN2", target_bir_lowering=False, debug=False)

    x_d = nc.dram_tensor("x", [B_L, I], F32, kind="ExternalInput").ap()
    wsp_d = nc.dram_tensor("wsp", [O, O], F32R, kind="ExternalInput").ap()
    sbase_d = nc.dram_tensor("sbase", [I, O], F32R, kind="ExternalInput").ap()
    fw_d = nc.dram_tensor("fw", [O, SU], F32R, kind="ExternalInput").ap()
    bw_d = nc.dram_tensor("bw", [O, S], F32R, kind="ExternalInput").ap()

    out_d = nc.dram_tensor("out", [B_L, S], F32, kind="ExternalOutput").ap()
    ab_d = nc.dram_tensor("ab", [B_L, SU], F32, kind="ExternalOutput").ap()
    bias_d = nc.dram_tensor("bias", [B_L, S], F32, kind="ExternalOutput").ap()

    with tile.TileContext(nc) as tc:
        with tc.tile_pool(name="consts", bufs=1) as consts, \
             tc.tile_pool(name="persist", bufs=1) as persist:
            ident = consts.tile([128, 128], F32)
            make_identity(nc, ident[:])
            rbias = consts.tile([128, 12], F32)
            for j in range(12):
                nc.vector.memset(rbias[:, j:j + 1], float(np.float32((5.5 - j) / CBRT6)))

            # x in natural layout [p, t, i]: row t*128+p of x
            x_nat = persist.tile([128, NBT, I], F32)
            nc.sync.dma_start(out=x_nat[:], in_=x_d.rearrange("(t p) i -> p t i", p=128))

            kan_sb = persist.tile([128, NOT, B_L], F32R)
            bias_sb = persist.tile([128, NBT, S], F32)
            out_acc = persist.tile([128, NBT, S], F32)

            # ---------------- Phase A: bases + KAN + bias ----------------
            with tc.tile_pool(name="wA", bufs=1) as wA, \
                 tc.tile_pool(name="xTp", bufs=1) as xTp, \
                 tc.tile_pool(name="bs", bufs=3) as bs, \
                 tc.tile_pool(name="bs2", bufs=2) as bs2, \
                 tc.tile_pool(name="bk", bufs=1) as bk, \
                 tc.tile_pool(name="psA", bufs=4, space="PSUM") as psA, \
                 tc.tile_pool(name="psT", bufs=2, space="PSUM") as psT, \
                 tc.tile_pool(name="psB", bufs=2, space="PSUM") as psB:

                wsp_sb = wA.tile([128, 8, O], F32R)
                nc.sync.dma_start(out=wsp_sb[:], in_=wsp_d.rearrange("(k i) o -> i k o", i=128))
                sbase_sb = wA.tile([128, O], F32R)
                nc.sync.dma_start(out=sbase_sb[:], in_=sbase_d)
                bw_sb = wA.tile([128, 8, S], F32R)
                nc.sync.dma_start(out=bw_sb[:], in_=bw_d.rearrange("(k i) s -> i k s", i=128))

                xT = xTp.tile([128, B_L], F32)
                for t in range(NBT):
                    ps_t = psT.tile([128, 128], F32)
                    nc.tensor.transpose(ps_t[:], x_nat[:, t, :], ident[:])
                    nc.scalar.copy(xT[:, t * 128:(t + 1) * 128], ps_t[:])

                siluT = xTp.tile([128, B_L], F32R)

                for ch in range(NCH):
                    sl = slice(ch * CH, (ch + 1) * CH)
                    nc.scalar.activation(siluT[:, sl], xT[:, sl], AF.Silu)

                    # bases: b_k = 4th finite difference of s_j = relu((t-j)/c)^3
                    s_l, d1_l, d2_l, d3_l, b_l = [], [], [], [], []
                    for j in range(12):
                        r = bs.tile([128, CH], F32, tag="r")
                        nc.scalar.activation(r[:], xT[:, sl], AF.Relu,
                                             bias=rbias[:, j:j + 1], scale=R_SCALE)
                        q = bs.tile([128, CH], F32, tag="q")
                        nc.vector.tensor_mul(q[:], r[:], r[:])
                        s_ = bs.tile([128, CH], F32, tag="s")
                        nc.vector.tensor_mul(s_[:], q[:], r[:])
                        s_l.append(s_)
                        if j >= 1:
                            d1 = bs2.tile([128, CH], F32, tag="d1")
                            nc.vector.tensor_sub(d1[:], s_l[j - 1][:], s_l[j][:])
                            d1_l.append(d1)
                        if j >= 2:
                            d2 = bs2.tile([128, CH], F32, tag="d2")
                            nc.vector.tensor_sub(d2[:], d1_l[j - 2][:], d1_l[j - 1][:])
                            d2_l.append(d2)
                        if j >= 3:
                            d3 = bs2.tile([128, CH], F32, tag="d3")
                            nc.vector.tensor_sub(d3[:], d2_l[j - 3][:], d2_l[j - 2][:])
                            d3_l.append(d3)
                        if j >= 4:
                            b_ = bk.tile([128, CH], F32R, tag=f"b{j - 4}")
                            nc.vector.tensor_sub(b_[:], d3_l[j - 4][:], d3_l[j - 3][:])
                            b_l.append(b_)

                    # kanT[o, b] = scale_base.T silu + sum_k wsp_k.T b_k
                    for ot in range(NOT):
                        osl = slice(ot * 128, (ot + 1) * 128)
                        ps = psA.tile([128, CH], F32)
                        nc.tensor.matmul(ps[:], sbase_sb[:, osl], siluT[:, sl],
                                         start=True, stop=False)
                        for k in range(8):
                            nc.tensor.matmul(ps[:], wsp_sb[:, k, osl], b_l[k][:],
                                             start=False, stop=(k == 7))
                        nc.scalar.copy(kan_sb[:, ot, sl], ps[:])

                    # bias[b, s] = kan.T @ bias_w for this chunk's 4 b-tiles
                    for t in range(ch * 4, ch * 4 + 4):
                        psb = psB.tile([128, S], F32)
                        for k in range(NOT):
                            nc.tensor.matmul(psb[:], kan_sb[:, k, t * 128:(t + 1) * 128],
                                             bw_sb[:, k, :], start=(k == 0), stop=(k == NOT - 1))
                        nc.vector.tensor_copy(bias_sb[:, t, :], psb[:])

            # ---------------- Phase B: AB + fused bmm ----------------
            with tc.tile_pool(name="fwp", bufs=2) as fwp, \
                 tc.tile_pool(name="abp", bufs=6) as abp, \
                 tc.tile_pool(name="dmp", bufs=4) as dmp, \
                 tc.tile_pool(name="psAB", bufs=6, space="PSUM") as psAB:
                fw_r = fw_d.rearrange("(k i) n -> i k n", i=128)
                for sc in range(NSC):
                    nsl = slice(sc * CH, (sc + 1) * CH)
                    fw_tiles = []
                    for k in range(NOT):
                        ft = fwp.tile([128, CH], F32R, tag=f"fw{k}")
                        nc.sync.dma_start(out=ft[:], in_=fw_r[:, k, nsl])
                        fw_tiles.append(ft)
                    for t in range(NBT):
                        ps = psAB.tile([128, CH], F32)
                        for k in range(NOT):
                            nc.tensor.matmul(ps[:], kan_sb[:, k, t * 128:(t + 1) * 128],
                                             fw_tiles[k][:], start=(k == 0), stop=(k == NOT - 1))
                        abt = abp.tile([128, CH], F32, tag="ab")
                        nc.scalar.copy(abt[:], ps[:])
                        nc.sync.dma_start(out=ab_d[t * 128:(t + 1) * 128, nsl], in_=abt[:])
                        for s4 in range(4):
                            dump = dmp.tile([128, 128], F32, tag="dump")
                            nc.vector.scalar_tensor_tensor(
                                out=dump[:], in0=ps[:, s4 * 128:(s4 + 1) * 128],
                                scalar=1.0, in1=x_nat[:, t, :],
                                op0=ALU.mult, op1=ALU.mult,
                                accum_out=out_acc[:, t, sc * 4 + s4: sc * 4 + s4 + 1])

            # ---------------- Final assembly ----------------
            with tc.tile_pool(name="fin", bufs=1) as fin:
                out_sb = fin.tile([128, NBT, S], F32)
                nc.vector.tensor_add(out_sb[:], out_acc[:], bias_sb[:])
                nc.sync.dma_start(out=out_d.rearrange("(t p) s -> p t s", p=128),
                                  in_=out_sb[:])
                nc.sync.dma_start(out=bias_d.rearrange("(t p) s -> p t s", p=128),
                                  in_=bias_sb[:])

    nc.compile()
    return nc


_NC_CACHE = []


def _get_nc():
    if not _NC_CACHE:
        _NC_CACHE.append(_build())
    return _NC_CACHE[0]


def kernel(inputs_novelU, inputs_state, coef, scale_base, scale_sp,
           bias_w, bias_b, f_w, f_b):
    nc = _get_nc()

    x = np.ascontiguousarray(
        np.concatenate([np.asarray(inputs_state), np.asarray(inputs_novelU)], axis=1),
        dtype=np.float32)
    wsp = np.ascontiguousarray(
        (np.asarray(scale_sp)[:, :, None] * np.asarray(coef)).transpose(2, 0, 1)
        .reshape(O, O), dtype=np.float32)
    sbase = np.ascontiguousarray(scale_base, dtype=np.float32)
    fw = np.ascontiguousarray(f_w, dtype=np.float32)
    bw = np.ascontiguousarray(bias_w, dtype=np.float32)

    in_maps = []
    for c in range(N_CORES):
        in_maps.append({
            "x": np.ascontiguousarray(x[c * B_L:(c + 1) * B_L]),
            "wsp": wsp, "sbase": sbase, "fw": fw, "bw": bw,
        })
    res = bass_utils.run_bass_kernel_spmd(nc, in_maps, core_ids=list(range(N_CORES)))

    out = np.concatenate([r["out"] for r in res.results], axis=0)
    AB = np.concatenate([r["ab"] for r in res.results], axis=0).reshape(B_FULL, S, I)
    bias = np.concatenate([r["bias"] for r in res.results], axis=0)

    # f_b / bias_b are zero in setup_inputs; fix up on host if ever nonzero.
    f_b = np.asarray(f_b)
    bias_b = np.asarray(bias_b)
    if f_b.any():
        AB = AB + f_b.reshape(1, S, I).astype(np.float32)
        out = out + x @ f_b.reshape(S, I).T
    if bias_b.any():
        bias = bias + bias_b.astype(np.float32)
        out = out + bias_b.astype(np.float32)

    return (out.astype(np.float32), AB.astype(np.float32), bias.astype(np.float32))


# revision 4
# speedup vs baseline: 1.0055x; 1.0055x over previous
"""Trainium2 Bass kernel for nn_BridgeNetworkKAN.

Math (per batch row b):
  x = concat(state, novelU)                                   [128]
  bases[i,c] = cubic B-spline basis of x_i on uniform grid    [128, 8]
  kan = silu(x) @ scale_base + einsum(bases, scale_sp*coef)   [1024]
  bias = kan @ bias_w (+ bias_b)                              [64]
  AB = (kan @ f_w (+ f_b)).reshape(64, 128)                   [64, 128]
  out = AB @ x + bias                                         [64]

Key tricks:
  * Uniform-knot B-spline bases via 4th finite difference of relu((t-j)/c)^3
    (c = cbrt(6), t = 2.5*x + 5.5): 12 ACT relus + 62 DVE TT ops per
    [128, 512] tile instead of the Cox-de-Boor recursion.
  * All matmuls in float32r (full PE rate for N=512; rel err ~2e-4).
  * Activations kept feature-on-partition (xT) so every contraction is a
    natural PE matmul; AB computed batch-on-partition so the 64 MB AB
    output DMAs out contiguously.
  * bmm (out = AB @ x) fused into the AB epilogue: 4 scalar_tensor_tensor
    ops per PSUM bank (multiply by x, accumulate-reduce over u) while the
    bank drains to SBUF on the scalar engine.

Sharding: data-parallel over batch, 16384/8 = 2048 rows per core;
all parameters replicated.
"""
import sys

if "/opt/trn_rl_repo" not in sys.path:
    sys.path.insert(0, "/opt/trn_rl_repo")

import numpy as np

import concourse.bass as bass
import concourse.mybir as mybir
import concourse.tile as tile
from concourse import bacc, bass_utils
from concourse.masks import make_identity

F32 = mybir.dt.float32
F32R = mybir.dt.float32r
AF = mybir.ActivationFunctionType
ALU = mybir.AluOpType

N_CORES = 8
B_FULL = 16384
B_L = B_FULL // N_CORES        # 2048 rows per core
CH = 512                       # batch chunk (matmul moving N)
NCH = B_L // CH                # 4
NBT = B_L // 128               # 16 batch tiles of 128
I = 128                        # input features (STATE + N_U)
O = 1024                       # KAN hidden
NOT = O // 128                 # 8 o-tiles
S = 64                         # STATE
SU = S * I                     # 8192
NSC = SU // CH                 # 16 su-chunks of 512

CBRT6 = float(np.cbrt(6.0))
R_SCALE = float(np.float32(2.5 / CBRT6))


def _build():
    nc = bacc.Bacc("TRN2", target_bir_lowering=False, debug=False)

    x_d = nc.dram_tensor("x", [B_L, I], F32, kind="ExternalInput").ap()
    wsp_d = nc.dram_tensor("wsp", [O, O], F32R, kind="ExternalInput").ap()
    sbase_d = nc.dram_tensor("sbase", [I, O], F32R, kind="ExternalInput").ap()
    fw_d = nc.dram_tensor("fw", [O, SU], F32R, kind="ExternalInput").ap()
    bw_d = nc.dram_tensor("bw", [O, S], F32R, kind="ExternalInput").ap()

    out_d = nc.dram_tensor("out", [B_L, S], F32, kind="ExternalOutput").ap()
    ab_d = nc.dram_tensor("ab", [B_L, SU], F32, kind="ExternalOutput").ap()
    bias_d = nc.dram_tensor("bias", [B_L, S], F32, kind="ExternalOutput").ap()

    with tile.TileContext(nc) as tc:
        with tc.tile_pool(name="consts", bufs=1) as consts, \
             tc.tile_pool(name="persist", bufs=1) as persist:
            ident = consts.tile([128, 128], F32)
            make_identity(nc, ident[:])
            rbias = consts.tile([128, 12], F32)
            for j in range(12):
                nc.vector.memset(rbias[:, j:j + 1], float(np.float32((5.5 - j) / CBRT6)))

            # x in natural layout [p, t, i]: row t*128+p of x
            x_nat = persist.tile([128, NBT, I], F32)
            nc.sync.dma_start(out=x_nat[:], in_=x_d.rearrange("(t p) i -> p t i", p=128))

            kan_sb = persist.tile([128, NOT, B_L], F32R)
            bias_sb = persist.tile([128, NBT, S], F32)
            out_acc = persist.tile([128, NBT, S], F32)

            # ---------------- Phase A: bases + KAN + bias ----------------
            with tc.tile_pool(name="wA", bufs=1) as wA, \
                 tc.tile_pool(name="xTp", bufs=1) as xTp, \
                 tc.tile_pool(name="bs", bufs=3) as bs, \
                 tc.tile_pool(name="bs2", bufs=2) as bs2, \
                 tc.tile_pool(name="bk", bufs=1) as bk, \
                 tc.tile_pool(name="psA", bufs=4, space="PSUM") as psA, \
                 tc.tile_pool(name="psT", bufs=2, space="PSUM") as psT, \
                 tc.tile_pool(name="psB", bufs=2, space="PSUM") as psB:

                wsp_sb = wA.tile([128, 8, O], F32R)
                nc.sync.dma_start(out=wsp_sb[:], in_=wsp_d.rearrange("(k i) o -> i k o", i=128))
                sbase_sb = wA.tile([128, O], F32R)
                nc.sync.dma_start(out=sbase_sb[:], in_=sbase_d)
                bw_sb = wA.tile([128, 8, S], F32R)
                nc.sync.dma_start(out=bw_sb[:], in_=bw_d.rearrange("(k i) s -> i k s", i=128))

                xT = xTp.tile([128, B_L], F32)
                for t in range(NBT):
                    ps_t = psT.tile([128, 128], F32)
                    nc.tensor.transpose(ps_t[:], x_nat[:, t, :], ident[:])
                    nc.scalar.copy(xT[:, t * 128:(t + 1) * 128], ps_t[:])

                siluT = xTp.tile([128, B_L], F32R)

                for ch in range(NCH):
                    sl = slice(ch * CH, (ch + 1) * CH)
                    nc.scalar.activation(siluT[:, sl], xT[:, sl], AF.Silu)

                    # bases: b_k = 4th finite difference of s_j = relu((t-j)/c)^3
                    s_l, d1_l, d2_l, d3_l, b_l = [], [], [], [], []
                    for j in range(12):
                        r = bs.tile([128, CH], F32, tag="r")
                        nc.scalar.activation(r[:], xT[:, sl], AF.Relu,
                                             bias=rbias[:, j:j + 1], scale=R_SCALE)
                        q = bs.tile([128, CH], F32, tag="q")
                        nc.scalar.activation(q[:], r[:], AF.Square)
                        s_ = bs.tile([128, CH], F32, tag="s")
                        nc.gpsimd.tensor_mul(s_[:], q[:], r[:])
                        s_l.append(s_)
                        if j >= 1:
                            d1 = bs2.tile([128, CH], F32, tag="d1")
                            nc.vector.tensor_sub(d1[:], s_l[j - 1][:], s_l[j][:])
                            d1_l.append(d1)
                        if j >= 2:
                            d2 = bs2.tile([128, CH], F32, tag="d2")
                            nc.vector.tensor_sub(d2[:], d1_l[j - 2][:], d1_l[j - 1][:])
                            d2_l.append(d2)
                        if j >= 3:
                            d3 = bs2.tile([128, CH], F32, tag="d3")
                            nc.vector.tensor_sub(d3[:], d2_l[j - 3][:], d2_l[j - 2][:])
                            d3_l.append(d3)
                        if j >= 4:
                            b_ = bk.tile([128, CH], F32R, tag=f"b{j - 4}")
                            nc.vector.tensor_sub(b_[:], d3_l[j - 4][:], d3_l[j - 3][:])
                            b_l.append(b_)

                    # kanT[o, b] = scale_base.T silu + sum_k wsp_k.T b_k
                    for ot in range(NOT):
                        osl = slice(ot * 128, (ot + 1) * 128)
                        ps = psA.tile([128, CH], F32)
                        nc.tensor.matmul(ps[:], sbase_sb[:, osl], siluT[:, sl],
                                         start=True, stop=False)
                        for k in range(8):
                            nc.tensor.matmul(ps[:], wsp_sb[:, k, osl], b_l[k][:],
                                             start=False, stop=(k == 7))
                        nc.scalar.copy(kan_sb[:, ot, sl], ps[:])

                    # bias[b, s] = kan.T @ bias_w for this chunk's 4 b-tiles
                    for t in range(ch * 4, ch * 4 + 4):
                        psb = psB.tile([128, S], F32)
                        for k in range(NOT):
                            nc.tensor.matmul(psb[:], kan_sb[:, k, t * 128:(t + 1) * 128],
                                             bw_sb[:, k, :], start=(k == 0), stop=(k == NOT - 1))
                        nc.vector.tensor_copy(bias_sb[:, t, :], psb[:])

            # ---------------- Phase B: AB + fused bmm ----------------
            with tc.tile_pool(name="fwp", bufs=2) as fwp, \
                 tc.tile_pool(name="abp", bufs=6) as abp, \
                 tc.tile_pool(name="dmp", bufs=4) as dmp, \
                 tc.tile_pool(name="psAB", bufs=6, space="PSUM") as psAB:
                fw_r = fw_d.rearrange("(k i) n -> i k n", i=128)
                for sc in range(NSC):
                    nsl = slice(sc * CH, (sc + 1) * CH)
                    fw_tiles = []
                    for k in range(NOT):
                        ft = fwp.tile([128, CH], F32R, tag=f"fw{k}")
                        nc.sync.dma_start(out=ft[:], in_=fw_r[:, k, nsl])
                        fw_tiles.append(ft)
                    for t in range(NBT):
                        ps = psAB.tile([128, CH], F32)
                        for k in range(NOT):
                            nc.tensor.matmul(ps[:], kan_sb[:, k, t * 128:(t + 1) * 128],
                                             fw_tiles[k][:], start=(k == 0), stop=(k == NOT - 1))
                        abt = abp.tile([128, CH], F32, tag="ab")
                        nc.scalar.copy(abt[:], ps[:])
                        nc.sync.dma_start(out=ab_d[t * 128:(t + 1) * 128, nsl], in_=abt[:])
                        # fused bmm: out[b, s] += sum_u AB0[b, s, u] * x[b, u]
                        prod = dmp.tile([128, CH], F32, tag="prod")
                        x_ap = x_nat[:, t, :]
                        x_bc = bass.AP(tensor=x_ap.tensor, offset=x_ap.offset,
                                       ap=[x_ap.ap[0], [0, 4], x_ap.ap[-1]])
                        nc.vector.tensor_tensor(
                            prod[:].rearrange("p (s u) -> p s u", s=4),
                            ps[:].rearrange("p (s u) -> p s u", s=4),
                            x_bc, op=ALU.mult)
                        nc.vector.tensor_reduce(
                            out=out_acc[:, t, sc * 4:(sc + 1) * 4],
                            in_=prod[:].rearrange("p (s u) -> p s u", s=4),
                            axis=mybir.AxisListType.X, op=ALU.add)

            # ---------------- Final assembly ----------------
            with tc.tile_pool(name="fin", bufs=1) as fin:
                out_sb = fin.tile([128, NBT, S], F32)
                nc.vector.tensor_add(out_sb[:], out_acc[:], bias_sb[:])
                nc.sync.dma_start(out=out_d.rearrange("(t p) s -> p t s", p=128),
                                  in_=out_sb[:])
                nc.sync.dma_start(out=bias_d.rearrange("(t p) s -> p t s", p=128),
                                  in_=bias_sb[:])

    nc.compile()
    return nc


_NC_CACHE = []


def _get_nc():
    if not _NC_CACHE:
        _NC_CACHE.append(_build())
    return _NC_CACHE[0]


def kernel(inputs_novelU, inputs_state, coef, scale_base, scale_sp,
           bias_w, bias_b, f_w, f_b):
    nc = _get_nc()

    x = np.ascontiguousarray(
        np.concatenate([np.asarray(inputs_state), np.asarray(inputs_novelU)], axis=1),
        dtype=np.float32)
    wsp = np.ascontiguousarray(
        (np.asarray(scale_sp)[:, :, None] * np.asarray(coef)).transpose(2, 0, 1)
        .reshape(O, O), dtype=np.float32)
    sbase = np.ascontiguousarray(scale_base, dtype=np.float32)
    fw = np.ascontiguousarray(f_w, dtype=np.float32)
    bw = np.ascontiguousarray(bias_w, dtype=np.float32)

    in_maps = []
    for c in range(N_CORES):
        in_maps.append({
            "x": np.ascontiguousarray(x[c * B_L:(c + 1) * B_L]),
            "wsp": wsp, "sbase": sbase, "fw": fw, "bw": bw,
        })
    res = bass_utils.run_bass_kernel_spmd(nc, in_maps, core_ids=list(range(N_CORES)))

    out = np.concatenate([r["out"] for r in res.results], axis=0)
    AB = np.concatenate([r["ab"] for r in res.results], axis=0).reshape(B_FULL, S, I)
    bias = np.concatenate([r["bias"] for r in res.results], axis=0)

    # f_b / bias_b are zero in setup_inputs; fix up on host if ever nonzero.
    f_b = np.asarray(f_b)
    bias_b = np.asarray(bias_b)
    if f_b.any():
        AB = AB + f_b.reshape(1, S, I).astype(np.float32)
        out = out + x @ f_b.reshape(S, I).T
    if bias_b.any():
        bias = bias + bias_b.astype(np.float32)
        out = out + bias_b.astype(np.float32)

    return (out.astype(np.float32), AB.astype(np.float32), bias.astype(np.float32))


# revision 5
# speedup vs baseline: 1.0071x; 1.0016x over previous
"""Trainium2 Bass kernel for nn_BridgeNetworkKAN.

Math (per batch row b):
  x = concat(state, novelU)                                   [128]
  bases[i,c] = cubic B-spline basis of x_i on uniform grid    [128, 8]
  kan = silu(x) @ scale_base + einsum(bases, scale_sp*coef)   [1024]
  bias = kan @ bias_w (+ bias_b)                              [64]
  AB = (kan @ f_w (+ f_b)).reshape(64, 128)                   [64, 128]
  out = AB @ x + bias                                         [64]

Key tricks:
  * Uniform-knot B-spline bases via 4th finite difference of relu((t-j)/c)^3
    (c = cbrt(6), t = 2.5*x + 5.5): 12 ACT relus + 62 DVE TT ops per
    [128, 512] tile instead of the Cox-de-Boor recursion.
  * All matmuls in float32r (full PE rate for N=512; rel err ~2e-4).
  * Activations kept feature-on-partition (xT) so every contraction is a
    natural PE matmul; AB computed batch-on-partition so the 64 MB AB
    output DMAs out contiguously.
  * bmm (out = AB @ x) fused into the AB epilogue: 4 scalar_tensor_tensor
    ops per PSUM bank (multiply by x, accumulate-reduce over u) while the
    bank drains to SBUF on the scalar engine.

Sharding: data-parallel over batch, 16384/8 = 2048 rows per core;
all parameters replicated.
"""
import sys

if "/opt/trn_rl_repo" not in sys.path:
    sys.path.insert(0, "/opt/trn_rl_repo")

import numpy as np

import concourse.bass as bass
import concourse.mybir as mybir
import concourse.tile as tile
from concourse import bacc, bass_utils
from concourse.masks import make_identity

F32 = mybir.dt.float32
F32R = mybir.dt.float32r
AF = mybir.ActivationFunctionType
ALU = mybir.AluOpType

N_CORES = 8
B_FULL = 16384
B_L = B_FULL // N_CORES        # 2048 rows per core
CH = 512                       # batch chunk (matmul moving N)
NCH = B_L // CH                # 4
NBT = B_L // 128               # 16 batch tiles of 128
I = 128                        # input features (STATE + N_U)
O = 1024                       # KAN hidden
NOT = O // 128                 # 8 o-tiles
S = 64                         # STATE
SU = S * I                     # 8192
NSC = SU // CH                 # 16 su-chunks of 512

CBRT6 = float(np.cbrt(6.0))
R_SCALE = float(np.float32(2.5 / CBRT6))


def _build():
    nc = bacc.Bacc("TRN2", target_bir_lowering=False, debug=False)

    x_d = nc.dram_tensor("x", [B_L, I], F32, kind="ExternalInput").ap()
    wsp_d = nc.dram_tensor("wsp", [O, O], F32R, kind="ExternalInput").ap()
    sbase_d = nc.dram_tensor("sbase", [I, O], F32R, kind="ExternalInput").ap()
    fw_d = nc.dram_tensor("fw", [O, SU], F32R, kind="ExternalInput").ap()
    bw_d = nc.dram_tensor("bw", [O, S], F32R, kind="ExternalInput").ap()

    out_d = nc.dram_tensor("out", [B_L, S], F32, kind="ExternalOutput").ap()
    ab_d = nc.dram_tensor("ab", [B_L, SU], F32, kind="ExternalOutput").ap()
    bias_d = nc.dram_tensor("bias", [B_L, S], F32, kind="ExternalOutput").ap()

    with tile.TileContext(nc) as tc:
        with tc.tile_pool(name="consts", bufs=1) as consts, \
             tc.tile_pool(name="persist", bufs=1) as persist:
            ident = consts.tile([128, 128], F32)
            make_identity(nc, ident[:])
            rbias = consts.tile([128, 12], F32)
            for j in range(12):
                nc.vector.memset(rbias[:, j:j + 1], float(np.float32((5.5 - j) / CBRT6)))

            # x in natural layout [p, t, i]: row t*128+p of x
            x_nat = persist.tile([128, NBT, I], F32)
            nc.sync.dma_start(out=x_nat[:], in_=x_d.rearrange("(t p) i -> p t i", p=128))

            kan_sb = persist.tile([128, NOT, B_L], F32R)
            bias_sb = persist.tile([128, NBT, S], F32)
            out_acc = persist.tile([128, NBT, S], F32)

            # ---------------- Phase A: bases + KAN + bias ----------------
            with tc.tile_pool(name="wA", bufs=1) as wA, \
                 tc.tile_pool(name="xTp", bufs=1) as xTp, \
                 tc.tile_pool(name="bs", bufs=3) as bs, \
                 tc.tile_pool(name="bs2", bufs=2) as bs2, \
                 tc.tile_pool(name="bk", bufs=1) as bk, \
                 tc.tile_pool(name="psA", bufs=4, space="PSUM") as psA, \
                 tc.tile_pool(name="psT", bufs=2, space="PSUM") as psT, \
                 tc.tile_pool(name="psB", bufs=2, space="PSUM") as psB:

                wsp_sb = wA.tile([128, 8, O], F32R)
                wsp_r = wsp_d.rearrange("(k i) o -> i k o", i=128)
                for k in range(8):
                    nc.sync.dma_start(out=wsp_sb[:, k, :], in_=wsp_r[:, k, :])
                sbase_sb = wA.tile([128, O], F32R)
                nc.sync.dma_start(out=sbase_sb[:], in_=sbase_d)
                bw_sb = wA.tile([128, 8, S], F32R)
                nc.sync.dma_start(out=bw_sb[:], in_=bw_d.rearrange("(k i) s -> i k s", i=128))

                xT = xTp.tile([128, B_L], F32)
                for t in range(NBT):
                    ps_t = psT.tile([128, 128], F32)
                    nc.tensor.transpose(ps_t[:], x_nat[:, t, :], ident[:])
                    nc.scalar.copy(xT[:, t * 128:(t + 1) * 128], ps_t[:])

                siluT = xTp.tile([128, B_L], F32R)

                for ch in range(NCH):
                    sl = slice(ch * CH, (ch + 1) * CH)
                    nc.scalar.activation(siluT[:, sl], xT[:, sl], AF.Silu)

                    # bases: b_k = 4th finite difference of s_j = relu((t-j)/c)^3
                    s_l, d1_l, d2_l, d3_l, b_l = [], [], [], [], []
                    for j in range(12):
                        r = bs.tile([128, CH], F32, tag="r")
                        nc.scalar.activation(r[:], xT[:, sl], AF.Relu,
                                             bias=rbias[:, j:j + 1], scale=R_SCALE)
                        q = bs.tile([128, CH], F32, tag="q")
                        nc.scalar.activation(q[:], r[:], AF.Square)
                        s_ = bs.tile([128, CH], F32, tag="s")
                        nc.gpsimd.tensor_mul(s_[:], q[:], r[:])
                        s_l.append(s_)
                        if j >= 1:
                            d1 = bs2.tile([128, CH], F32, tag="d1")
                            nc.vector.tensor_sub(d1[:], s_l[j - 1][:], s_l[j][:])
                            d1_l.append(d1)
                        if j >= 2:
                            d2 = bs2.tile([128, CH], F32, tag="d2")
                            nc.vector.tensor_sub(d2[:], d1_l[j - 2][:], d1_l[j - 1][:])
                            d2_l.append(d2)
                        if j >= 3:
                            d3 = bs2.tile([128, CH], F32, tag="d3")
                            nc.vector.tensor_sub(d3[:], d2_l[j - 3][:], d2_l[j - 2][:])
                            d3_l.append(d3)
                        if j >= 4:
                            b_ = bk.tile([128, CH], F32R, tag=f"b{j - 4}")
                            nc.vector.tensor_sub(b_[:], d3_l[j - 4][:], d3_l[j - 3][:])
                            b_l.append(b_)

                    # kanT[o, b] = scale_base.T silu + sum_k wsp_k.T b_k
                    for ot in range(NOT):
                        osl = slice(ot * 128, (ot + 1) * 128)
                        ps = psA.tile([128, CH], F32)
                        nc.tensor.matmul(ps[:], sbase_sb[:, osl], siluT[:, sl],
                                         start=True, stop=False)
                        for k in range(8):
                            nc.tensor.matmul(ps[:], wsp_sb[:, k, osl], b_l[k][:],
                                             start=False, stop=(k == 7))
                        nc.scalar.copy(kan_sb[:, ot, sl], ps[:])

                    # bias[b, s] = kan.T @ bias_w for this chunk's 4 b-tiles
                    for t in range(ch * 4, ch * 4 + 4):
                        psb = psB.tile([128, S], F32)
                        for k in range(NOT):
                            nc.tensor.matmul(psb[:], kan_sb[:, k, t * 128:(t + 1) * 128],
                                             bw_sb[:, k, :], start=(k == 0), stop=(k == NOT - 1))
                        nc.vector.tensor_copy(bias_sb[:, t, :], psb[:])

            # ---------------- Phase B: AB + fused bmm ----------------
            with tc.tile_pool(name="fwp", bufs=3) as fwp, \
                 tc.tile_pool(name="abp", bufs=8) as abp, \
                 tc.tile_pool(name="dmp", bufs=4) as dmp, \
                 tc.tile_pool(name="psAB", bufs=8, space="PSUM") as psAB:
                fw_r = fw_d.rearrange("(k i) n -> i k n", i=128)
                for sc in range(NSC):
                    nsl = slice(sc * CH, (sc + 1) * CH)
                    fw_tiles = []
                    for k in range(NOT):
                        ft = fwp.tile([128, CH], F32R, tag=f"fw{k}")
                        nc.sync.dma_start(out=ft[:], in_=fw_r[:, k, nsl])
                        fw_tiles.append(ft)
                    for t in range(NBT):
                        ps = psAB.tile([128, CH], F32)
                        for k in range(NOT):
                            nc.tensor.matmul(ps[:], kan_sb[:, k, t * 128:(t + 1) * 128],
                                             fw_tiles[k][:], start=(k == 0), stop=(k == NOT - 1))
                        abt = abp.tile([128, CH], F32, tag="ab")
                        nc.scalar.copy(abt[:], ps[:])
                        nc.sync.dma_start(out=ab_d[t * 128:(t + 1) * 128, nsl], in_=abt[:])
                        # fused bmm: out[b, s] += sum_u AB0[b, s, u] * x[b, u]
                        prod = dmp.tile([128, CH], F32, tag="prod")
                        x_ap = x_nat[:, t, :]
                        x_bc = bass.AP(tensor=x_ap.tensor, offset=x_ap.offset,
                                       ap=[x_ap.ap[0], [0, 4], x_ap.ap[-1]])
                        nc.vector.tensor_tensor(
                            prod[:].rearrange("p (s u) -> p s u", s=4),
                            ps[:].rearrange("p (s u) -> p s u", s=4),
                            x_bc, op=ALU.mult)
                        nc.vector.tensor_reduce(
                            out=out_acc[:, t, sc * 4:(sc + 1) * 4],
                            in_=prod[:].rearrange("p (s u) -> p s u", s=4),
                            axis=mybir.AxisListType.X, op=ALU.add)

            # ---------------- Final assembly ----------------
            with tc.tile_pool(name="fin", bufs=1) as fin:
                out_sb = fin.tile([128, NBT, S], F32)
                nc.vector.tensor_add(out_sb[:], out_acc[:], bias_sb[:])
                nc.sync.dma_start(out=out_d.rearrange("(t p) s -> p t s", p=128),
                                  in_=out_sb[:])
                nc.sync.dma_start(out=bias_d.rearrange("(t p) s -> p t s", p=128),
                                  in_=bias_sb[:])

    nc.compile()
    return nc


_NC_CACHE = []


def _get_nc():
    if not _NC_CACHE:
        _NC_CACHE.append(_build())
    return _NC_CACHE[0]


def kernel(inputs_novelU, inputs_state, coef, scale_base, scale_sp,
           bias_w, bias_b, f_w, f_b):
    nc = _get_nc()

    x = np.ascontiguousarray(
        np.concatenate([np.asarray(inputs_state), np.asarray(inputs_novelU)], axis=1),
        dtype=np.float32)
    wsp = np.ascontiguousarray(
        (np.asarray(scale_sp)[:, :, None] * np.asarray(coef)).transpose(2, 0, 1)
        .reshape(O, O), dtype=np.float32)
    sbase = np.ascontiguousarray(scale_base, dtype=np.float32)
    fw = np.ascontiguousarray(f_w, dtype=np.float32)
    bw = np.ascontiguousarray(bias_w, dtype=np.float32)

    in_maps = []
    for c in range(N_CORES):
        in_maps.append({
            "x": np.ascontiguousarray(x[c * B_L:(c + 1) * B_L]),
            "wsp": wsp, "sbase": sbase, "fw": fw, "bw": bw,
        })
    res = bass_utils.run_bass_kernel_spmd(nc, in_maps, core_ids=list(range(N_CORES)))

    out = np.concatenate([r["out"] for r in res.results], axis=0)
    AB = np.concatenate([r["ab"] for r in res.results], axis=0).reshape(B_FULL, S, I)
    bias = np.concatenate([r["bias"] for r in res.results], axis=0)

    # f_b / bias_b are zero in setup_inputs; fix up on host if ever nonzero.
    f_b = np.asarray(f_b)
    bias_b = np.asarray(bias_b)
    if f_b.any():
        AB = AB + f_b.reshape(1, S, I).astype(np.float32)
        out = out + x @ f_b.reshape(S, I).T
    if bias_b.any():
        bias = bias + bias_b.astype(np.float32)
        out = out + bias_b.astype(np.float32)

    return (out.astype(np.float32), AB.astype(np.float32), bias.astype(np.float32))
